# revision 1
# baseline (speedup 1.0000x reference)
"""AttentiveConv TRN2 kernel: out = (softmax_n((text@We)@ctx^T) @ ctx) @ W2^T.

Sharded data-parallel over batch B=8 across 8 NeuronCores (one batch each).
Inputs are pre-transposed / fp32r-pre-rounded on host; matmuls run in fp32r
(TF32, 1 cyc/row — 4x faster than fp32) except mm3 which runs in bf16 (the
output error is dominated by the scores path, so bf16 attn costs nothing).

Per-core dataflow (PSUM accumulates fp32 throughout):
  A: tempT[D,N]   = matmul(lhsT=We[d',d],      rhs=textT[d',n])       fp32r
  B: scoresT[M,N] = matmul(lhsT=ctxT[d',m],    rhs=tempT[d',n])       fp32r
     softmax along the free axis n per 128-row m-tile:
     attn = exp(s - max_n)/Z (exp+Z in one ACT pass), attn -> bf16 DRAM spill
  C: resT[D,N]    = matmul(lhsT=ctx[m,d'],     rhs=attnT[m,n])        bf16
  D: out[N,D]     = matmul(lhsT=resT[d',n],    rhs=W2T[d',d])         fp32r

scoresT layout (scores transposed) makes the softmax axis the free axis and
every matmul consume its predecessor's natural output layout. attn round-trips
through DRAM because softmax-over-queries prevents flash-style fusion (the
softmax axis N differs from mm3's contraction axis M) and full attn (16MB
fp32r / 8MB bf16) cannot stay in SBUF next to ctx/tempT.

Measured: ~335-365 us/core steady-state on HW (PE roofline 327 us; 25.8
GFLOP/core at 78.6 TFLOP/s); relative error vs fp32 reference 3.6e-3.
"""

import sys

sys.path.insert(0, "/opt/trn_rl_repo")

from contextlib import ExitStack

import ml_dtypes
import numpy as np

B, N, M, D = 8, 2048, 2048, 1024
P = 128
KT = D // P  # 8 contraction tiles for d'
MT = M // P  # 16 m-tiles
NCH_A = 512  # phase A n-chunk
NCH_C = 256  # phase C/D n-chunk
SPLIT_WE = True  # split initial We load per output-column block
SPLIT_C_LOADS = True  # per-mt attn loads in phase C
PSA_BUFS = 4
PSB_BUFS = 8
CTP_BUFS = 4
EPL_BUFS = 2
PSC_BUFS = 6
TXP_BUFS = 2
ATP_BUFS = 2
ACP_BUFS = 4
KEEP_ATTN = True
BIG_PSUM_B = True  # one 4-bank [128,2048] psum tile per m-tile in phase B
D_EVICT_ACT = True  # route phase D psum evictions to ScalarE (ACT idle in C/D)  # SBUF-kept attn tiles hurt replica pipelining; DRAM RT is fine

_cache = {}


def r11(x: np.ndarray) -> np.ndarray:
    """Round fp32 to fp32r (TF32: 11 explicit mantissa bits, round-nearest-even)."""
    x = np.ascontiguousarray(x, dtype=np.float32)
    u = x.view(np.uint32).astype(np.uint64)
    bias = ((u >> 12) & 1) + 0x7FF
    u = (u + bias) & np.uint64(0xFFFFF000)
    return u.astype(np.uint32).view(np.float32).reshape(x.shape)


def _build(replicas=1, phases="ABCD"):
    """replicas>1 repeats the whole pipeline in one NEFF (for HW timing
    amortization); phases subsets the pipeline (for attribution)."""
    import concourse.bass as bass  # noqa: F401
    import concourse.mybir as mybir
    import concourse.tile as tile
    from concourse import bacc

    f32 = mybir.dt.float32
    f32r = mybir.dt.float32r

    nc = bacc.Bacc(None, target_bir_lowering=False)

    textT_d = nc.declare_dram_parameter("textT", [D, N], f32r, isOutput=False)
    ctxT_d = nc.declare_dram_parameter("ctxT", [MT, P, KT, P], f32r, isOutput=False)
    ctx_d = nc.declare_dram_parameter("ctx", [M, D], mybir.dt.bfloat16, isOutput=False)
    we_d = nc.declare_dram_parameter("we", [D, D], f32r, isOutput=False)
    w2T_d = nc.declare_dram_parameter("w2T", [D, D], f32r, isOutput=False)
    out_d = nc.declare_dram_parameter("out", [N, D], f32, isOutput=True)
    attn_sc = nc.dram_tensor("attn_sc", [MT, P, N], mybir.dt.bfloat16)

    with tile.TileContext(nc) as tc, ExitStack() as top:
        # whole-kernel residents: We/W2T/ctx are replica-invariant, so they
        # load ONCE before the replica loop. This removes 12MB/replica of
        # weight DMA (which head-of-line-blocked phase A/B tile streams on
        # the shared DMA queue) and the We<->W2T slab conflict that
        # serialized consecutive replicas.
        consts = top.enter_context(tc.tile_pool(name="consts", bufs=1))
        ctx_sb = consts.tile([P, MT, D], mybir.dt.bfloat16)  # 32KB/p, phase C lhsT (bf16)
        weslab = consts.tile([P, KT, D], f32r)  # 32KB/p: We (phase A lhsT)
        w2slab = consts.tile([P, KT, D], f32r)  # 32KB/p: W2T (phase D lhsT)

        if "A" in phases:
            nc.sync.dma_start(
                weslab[:], we_d[:].rearrange("(kt p) d -> p kt d", p=P)
            )
        if "C" in phases:
            nc.sync.dma_start(
                ctx_sb[:], ctx_d[:].rearrange("(mt p) d -> p mt d", p=P)
            )
        if "D" in phases:
            nc.sync.dma_start(
                w2slab[:], w2T_d[:].rearrange("(kt p) d -> p kt d", p=P)
            )

        for _rep in range(replicas):
            _emit_pipeline(
                nc, tc, mybir, f32, f32r, phases, ctx_sb, weslab, w2slab,
                textT_d, ctxT_d, ctx_d, we_d, w2T_d, out_d, attn_sc,
            )

    nc.compile()
    return nc


def _emit_pipeline(
    nc, tc, mybir, f32, f32r, phases, ctx_sb, weslab, w2slab,
    textT_d, ctxT_d, ctx_d, we_d, w2T_d, out_d, attn_sc,
):
    from contextlib import ExitStack

    with ExitStack() as rep_stack:
        keep_attn = {}
        # attn tiles for the last ATP_BUFS m-tiles stay resident into phase C
        # (skips their DRAM round-trip on the critical path)
        atp = rep_stack.enter_context(tc.tile_pool(name="atp", bufs=ATP_BUFS))

        with ExitStack() as ab_stack:
            tempT_pool = ab_stack.enter_context(tc.tile_pool(name="tempT", bufs=1))
            tempT = tempT_pool.tile([P, KT, N], f32r)  # 64KB/p

            # ---- Phase A: tempT = We.T-layout matmul over textT ----
            with ExitStack() as a_stack:
              if "A" in phases:
                txp = a_stack.enter_context(tc.tile_pool(name="txp", bufs=TXP_BUFS))
                psA = a_stack.enter_context(
                    tc.tile_pool(name="psA", bufs=PSA_BUFS, space="PSUM")
                )
                textT_ap = textT_d[:].rearrange("(kt p) n -> p kt n", p=P)
                for ch in range(N // NCH_A):
                    tx = txp.tile([P, KT, NCH_A], f32r)
                    nc.sync.dma_start(
                        tx[:], textT_ap[:, :, ch * NCH_A : (ch + 1) * NCH_A]
                    )
                    for dt in range(KT):
                        ps = psA.tile([P, NCH_A], f32)
                        for kt in range(KT):
                            nc.tensor.matmul(
                                ps[:],
                                weslab[:, kt, dt * P : (dt + 1) * P],
                                tx[:, kt],
                                start=(kt == 0),
                                stop=(kt == KT - 1),
                            )
                        nc.vector.tensor_copy(
                            tempT[:, dt, ch * NCH_A : (ch + 1) * NCH_A], ps[:]
                        )

            # ---- Phase B: scoresT per m-tile + softmax over n, spill attn ----
            with ExitStack() as b_stack:
              if "B" in phases:
                ctp = b_stack.enter_context(tc.tile_pool(name="ctp", bufs=CTP_BUFS))
                psB = b_stack.enter_context(
                    tc.tile_pool(
                        name="psB",
                        bufs=2 if BIG_PSUM_B else PSB_BUFS,
                        space="PSUM",
                    )
                )
                smp = b_stack.enter_context(tc.tile_pool(name="smp", bufs=4))
                epl = b_stack.enter_context(tc.tile_pool(name="epl", bufs=EPL_BUFS))
                NJ = 512
                for mt in range(MT):
                    ctm = ctp.tile([P, KT, P], f32r)
                    nc.sync.dma_start(ctm[:], ctxT_d[mt])
                    if BIG_PSUM_B:
                        psw = psB.tile([P, N], f32, tag="psB", name="psB")
                        for j in range(N // NJ):
                            for kt in range(KT):
                                nc.tensor.matmul(
                                    psw[:, j * NJ : (j + 1) * NJ],
                                    ctm[:, kt],
                                    tempT[:, kt, j * NJ : (j + 1) * NJ],
                                    start=(kt == 0),
                                    stop=(kt == KT - 1),
                                )
                        nmax = smp.tile([P, 1], f32)
                        nc.vector.reduce_max(
                            nmax[:], psw[:], axis=mybir.AxisListType.X, negate=True
                        )
                        attn = atp.tile([P, N], mybir.dt.bfloat16)
                        esb = epl.tile([P, N], f32)
                        z = smp.tile([P, 1], f32)
                        nc.scalar.activation(
                            esb[:],
                            psw[:],
                            mybir.ActivationFunctionType.Exp,
                            bias=nmax[:],
                            accum_out=z[:],
                        )
                        zinv = smp.tile([P, 1], f32)
                        nc.vector.reciprocal(zinv[:], z[:])
                        nc.vector.tensor_scalar_mul(attn[:], esb[:], zinv[:])
                        keep_attn[mt] = attn
                        if not KEEP_ATTN or mt < MT - ATP_BUFS:
                            nc.sync.dma_start(attn_sc[mt], attn[:])
                        continue
                    pss = []
                    for j in range(N // NJ):
                        ps = psB.tile([P, NJ], f32, tag="psBs", name="psBs")
                        for kt in range(KT):
                            nc.tensor.matmul(
                                ps[:],
                                ctm[:, kt],
                                tempT[:, kt, j * NJ : (j + 1) * NJ],
                                start=(kt == 0),
                                stop=(kt == KT - 1),
                            )
                        pss.append(ps)
                    nm4 = smp.tile([P, 4], f32)
                    for j, ps in enumerate(pss):
                        nc.vector.reduce_max(
                            nm4[:, j : j + 1], ps[:], axis=mybir.AxisListType.X
                        )
                    nmax = smp.tile([P, 1], f32)
                    nc.vector.reduce_max(
                        nmax[:], nm4[:], axis=mybir.AxisListType.X, negate=True
                    )
                    attn = atp.tile([P, N], mybir.dt.bfloat16)
                    esb = epl.tile([P, N], f32)
                    zp4 = smp.tile([P, 4], f32)
                    for j, ps in enumerate(pss):
                        nc.scalar.activation(
                            esb[:, j * NJ : (j + 1) * NJ],
                            ps[:],
                            mybir.ActivationFunctionType.Exp,
                            bias=nmax[:],
                            accum_out=zp4[:, j : j + 1],
                        )
                    z = smp.tile([P, 1], f32)
                    nc.vector.reduce_sum(z[:], zp4[:], axis=mybir.AxisListType.X)
                    zinv = smp.tile([P, 1], f32)
                    nc.vector.reciprocal(zinv[:], z[:])
                    nc.vector.tensor_scalar_mul(attn[:], esb[:], zinv[:])
                    keep_attn[mt] = attn
                    if not KEEP_ATTN or mt < MT - ATP_BUFS:
                        nc.sync.dma_start(attn_sc[mt], attn[:])

        # ---- Phase C+D: resT accum over m, then out = resT.T @ W2T ----
        with ExitStack() as cd_stack:
          if "C" in phases:
            acp = cd_stack.enter_context(tc.tile_pool(name="acp", bufs=ACP_BUFS))
            rtp = cd_stack.enter_context(tc.tile_pool(name="rtp", bufs=2))
            outp = cd_stack.enter_context(tc.tile_pool(name="outp", bufs=2))
            psC = cd_stack.enter_context(tc.tile_pool(name="psC", bufs=PSC_BUFS, space="PSUM"))
            psD = cd_stack.enter_context(tc.tile_pool(name="psD", bufs=2, space="PSUM"))
            attn_ap = attn_sc[:].rearrange("mt p n -> p mt n")
            for ch in range(N // NCH_C):
                ach = acp.tile([P, MT, NCH_C], mybir.dt.bfloat16)
                # per-mt loads so C's accumulation chases B's per-mt attn
                # writes instead of waiting for all of B to finish
                n_dram_mt = MT - ATP_BUFS if KEEP_ATTN else MT
                if SPLIT_C_LOADS:
                    for mt in range(n_dram_mt):
                        nc.sync.dma_start(
                            ach[:, mt],
                            attn_ap[:, mt, ch * NCH_C : (ch + 1) * NCH_C],
                        )
                else:
                    nc.sync.dma_start(
                        ach[:, :n_dram_mt],
                        attn_ap[:, :n_dram_mt, ch * NCH_C : (ch + 1) * NCH_C],
                    )
                resT = rtp.tile([P, KT, NCH_C], f32r)
                for g in range(2):
                    pss = []
                    for i in range(4):
                        pst = psC.tile([P, NCH_C], f32, tag="psC", name="psC")
                        pss.append(pst)
                    for mt in range(MT):
                        if KEEP_ATTN and mt >= MT - ATP_BUFS:
                            rhs = keep_attn[mt][:, ch * NCH_C : (ch + 1) * NCH_C]
                        else:
                            rhs = ach[:, mt]
                        for i in range(4):
                            dtt = g * 4 + i
                            nc.tensor.matmul(
                                pss[i][:],
                                ctx_sb[:, mt, dtt * P : (dtt + 1) * P],
                                rhs,
                                start=(mt == 0),
                                stop=(mt == MT - 1),
                            )
                    for i in range(4):
                        nc.vector.tensor_copy(resT[:, g * 4 + i], pss[i][:])
                for nb in range(NCH_C // P if "D" in phases else 0):
                    osb = outp.tile([P, D], f32)
                    for dc in range(2):
                        ps = psD.tile([P, 512], f32, tag="psD", name="psD")
                        for dtt in range(KT):
                            nc.tensor.matmul(
                                ps[:],
                                resT[:, dtt, nb * P : (nb + 1) * P],
                                w2slab[:, dtt, dc * 512 : (dc + 1) * 512],
                                start=(dtt == 0),
                                stop=(dtt == KT - 1),
                            )
                        if D_EVICT_ACT:
                            nc.scalar.copy(osb[:, dc * 512 : (dc + 1) * 512], ps[:])
                        else:
                            nc.vector.tensor_copy(
                                osb[:, dc * 512 : (dc + 1) * 512], ps[:]
                            )
                    row0 = ch * NCH_C + nb * P
                    nc.scalar.dma_start(out_d[:][row0 : row0 + P, :], osb[:])

    nc.compile()
    return nc


def _prep_inputs(text, context, We, W2):
    """Per-core host-side shard + transpose + fp32r pre-round."""
    we_r = r11(We)
    w2T_r = r11(W2.T)
    maps = []
    for b in range(B):
        maps.append(
            {
                "textT": r11(text[b].T),
                "ctxT": np.ascontiguousarray(
                    r11(context[b].T).reshape(KT, P, MT, P).transpose(2, 1, 0, 3)
                ),
                "ctx": context[b].astype(ml_dtypes.bfloat16),
                "we": we_r,
                "w2T": w2T_r,
            }
        )
    return maps


def kernel(text, context, We, W2, _trace=False):
    from concourse.bass_utils import run_bass_kernel_spmd

    if "nc" not in _cache:
        _cache["nc"] = _build()
    nc = _cache["nc"]
    in_maps = _prep_inputs(
        np.asarray(text), np.asarray(context), np.asarray(We), np.asarray(W2)
    )
    res = run_bass_kernel_spmd(nc, in_maps, list(range(B)), trace=_trace)
    out = np.stack([res.results[c]["out"] for c in range(B)])
    if _trace:
        return out, res
    return out



# revision 2
# speedup vs baseline: 1.0662x; 1.0662x over previous
"""AttentiveConv TRN2 kernel: out = (softmax_n((text@We)@ctx^T) @ ctx) @ W2^T.

Sharded data-parallel over batch B=8 across 8 NeuronCores (one batch each).
Matmul dtype strategy:
  A, B (score path) run fp32r (TF32, 1 cyc/row): the softmax amplifies score
  errors (logit std ~32), so the score path needs ~11 mantissa bits.
  C, D (value path) run fp8-e4m3 in DoubleRow perf mode (K=256 per matmul,
  0.5 cyc/row = 4x the f32r MAC rate), with each operand split hi/lo
  (x = f8(x) + f8(x - f8(x))) and 3 of the 4 cross terms kept
  (hi*hi + hi*lo + lo*hi): ~2^-8.5 effective precision at 0.75x the bf16
  PE cost. Scales keep fp8 operands in the normal range: attn*64, res/16,
  w2*32 (net factor 2 folded out of the final PSUM eviction).

Per-core dataflow (PSUM accumulates fp32 throughout):
  A: tempT[D,N]   = matmul(lhsT=We[d',d],      rhs=textT[d',n])       fp32r
  B: scoresT[M,N] = matmul(lhsT=ctxT[d',m],    rhs=tempT[d',n])       fp32r
     softmax along the free axis n per 128-row m-tile:
     hi = f8(exp(s-max)*64/Z) via ACT (scale=64/Z), lo via one fused DVE
     scalar_tensor_tensor; hi/lo spill to DRAM per 256-row m-pair.
  C: resT[D,N]    = DoubleRow matmuls over m-pairs (3 hi/lo passes)    fp8
  D: out[N,D]     = DoubleRow matmuls over d'-pairs (3 hi/lo passes)   fp8

PE roofline: A 131072 + B 262144 + C 196608 + D 98304 = 688128 cyc @2.4GHz
= 286.7 us/core (25.8 GFLOP). Relative error vs fp32 reference ~5e-3.
"""

import sys

sys.path.insert(0, "/opt/trn_rl_repo")

from contextlib import ExitStack

import ml_dtypes
import numpy as np

B, N, M, D = 8, 2048, 2048, 1024
P = 128
KT = D // P  # 8 contraction tiles for d'
MT = M // P  # 16 m-tiles (phase B)
MT2 = M // (2 * P)  # 8 m-pairs (phase C DoubleRow k-tiles)
DT2 = D // (2 * P)  # 4 d'-pairs (phase D DoubleRow k-tiles)
NCH_A = 512  # phase A n-chunk
NCH_C = 512  # phase C/D n-chunk
ALPHA = 64.0  # attn fp8 scale
W2S = 32.0  # w2 fp8 scale
C1 = 1.0 / 1024.0  # res eviction scale: psC*(1/(ALPHA*16)) = res/16
DOUT = 0.5  # phase D eviction scale: (res/16)*(32*w2) = 2*out
PSA_BUFS = 4
PSB_BUFS = 2
CTP_BUFS = 4
EPL_BUFS = 2
PSC_BUFS = 6
TXP_BUFS = 2
ACP_BUFS = 3
KEEP_PAIRS = 1  # last m-pairs kept in SBUF into phase C (skip DRAM RT)

_cache = {}


def r11(x: np.ndarray) -> np.ndarray:
    """Round fp32 to fp32r (TF32: 11 explicit mantissa bits, round-nearest-even)."""
    x = np.ascontiguousarray(x, dtype=np.float32)
    u = x.view(np.uint32).astype(np.uint64)
    bias = ((u >> 12) & 1) + 0x7FF
    u = (u + bias) & np.uint64(0xFFFFF000)
    return u.astype(np.uint32).view(np.float32).reshape(x.shape)


def _split8(x: np.ndarray):
    """hi/lo fp8-e4m3 split: x ~= hi + lo."""
    hi = x.astype(ml_dtypes.float8_e4m3)
    lo = (x - hi.astype(np.float32)).astype(ml_dtypes.float8_e4m3)
    return hi, lo


def _pair_layout(a: np.ndarray, ktiles: int) -> np.ndarray:
    """[K, D] -> [P, ktiles, 2, D] with K = ktiles*2*P (DoubleRow lhsT/rhs)."""
    k, d = a.shape
    assert k == ktiles * 2 * P
    return np.ascontiguousarray(a.reshape(ktiles, 2, P, d).transpose(2, 0, 1, 3))


def _build(replicas=1, phases="ABCD"):
    """replicas>1 repeats the whole pipeline in one NEFF (for HW timing
    amortization); phases subsets the pipeline (for attribution)."""
    import concourse.bass as bass  # noqa: F401
    import concourse.mybir as mybir
    import concourse.tile as tile
    from concourse import bacc

    f32 = mybir.dt.float32
    f32r = mybir.dt.float32r
    f8 = mybir.dt.float8e4

    nc = bacc.Bacc(None, target_bir_lowering=False)

    textT_d = nc.declare_dram_parameter("textT", [D, N], f32r, isOutput=False)
    ctxT_d = nc.declare_dram_parameter("ctxT", [MT, P, KT, P], f32r, isOutput=False)
    ctxh_d = nc.declare_dram_parameter("ctxh", [P, MT2, 2, D], f8, isOutput=False)
    ctxl_d = nc.declare_dram_parameter("ctxl", [P, MT2, 2, D], f8, isOutput=False)
    we_d = nc.declare_dram_parameter("we", [D, D], f32r, isOutput=False)
    w2h_d = nc.declare_dram_parameter("w2h", [P, DT2, 2, D], f8, isOutput=False)
    w2l_d = nc.declare_dram_parameter("w2l", [P, DT2, 2, D], f8, isOutput=False)
    out_d = nc.declare_dram_parameter("out", [N, D], f32, isOutput=True)
    attnh_sc = nc.dram_tensor("attnh_sc", [MT2, P, 2, N], f8)
    attnl_sc = nc.dram_tensor("attnl_sc", [MT2, P, 2, N], f8)

    with tile.TileContext(nc) as tc, ExitStack() as top:
        # whole-kernel residents: We/W2/ctx slabs are replica-invariant and
        # load once before the replica loop.
        consts = top.enter_context(tc.tile_pool(name="consts", bufs=1))
        ctxh_sb = consts.tile([P, MT2, 2, D], f8)  # 16KB/p, phase C lhsT hi
        ctxl_sb = consts.tile([P, MT2, 2, D], f8)  # 16KB/p, phase C lhsT lo
        weslab = consts.tile([P, KT, D], f32r)  # 32KB/p: We (phase A lhsT)
        w2h_sb = consts.tile([P, DT2, 2, D], f8)  # 8KB/p: 32*W2T hi (D rhs)
        w2l_sb = consts.tile([P, DT2, 2, D], f8)  # 8KB/p: 32*W2T lo (D rhs)

        if "A" in phases:
            nc.sync.dma_start(
                weslab[:], we_d[:].rearrange("(kt p) d -> p kt d", p=P)
            )
        if "C" in phases:
            nc.sync.dma_start(ctxh_sb[:], ctxh_d[:])
            nc.sync.dma_start(ctxl_sb[:], ctxl_d[:])
        if "D" in phases:
            nc.sync.dma_start(w2h_sb[:], w2h_d[:])
            nc.sync.dma_start(w2l_sb[:], w2l_d[:])

        for _rep in range(replicas):
            _emit_pipeline(
                nc, tc, mybir, f32, f32r, f8, phases, ctxh_sb, ctxl_sb, weslab,
                w2h_sb, w2l_sb, textT_d, ctxT_d, out_d, attnh_sc, attnl_sc,
            )

    nc.compile()
    return nc


def _emit_pipeline(
    nc, tc, mybir, f32, f32r, f8, phases, ctxh_sb, ctxl_sb, weslab,
    w2h_sb, w2l_sb, textT_d, ctxT_d, out_d, attnh_sc, attnl_sc,
):
    from contextlib import ExitStack

    mult = mybir.AluOpType.mult
    subtract = mybir.AluOpType.subtract

    with ExitStack() as rep_stack:
        keep_pairs = {}
        # attn hi/lo tiles for the last KEEP_PAIRS m-pairs stay resident into
        # phase C (skips their DRAM round-trip on the critical path)
        atp = rep_stack.enter_context(tc.tile_pool(name="atp", bufs=2))

        with ExitStack() as ab_stack:
            tempT_pool = ab_stack.enter_context(tc.tile_pool(name="tempT", bufs=1))
            tempT = tempT_pool.tile([P, KT, N], f32r)  # 64KB/p

            # ---- Phase A: tempT = We.T-layout matmul over textT ----
            with ExitStack() as a_stack:
              if "A" in phases:
                txp = a_stack.enter_context(tc.tile_pool(name="txp", bufs=TXP_BUFS))
                psA = a_stack.enter_context(
                    tc.tile_pool(name="psA", bufs=PSA_BUFS, space="PSUM")
                )
                textT_ap = textT_d[:].rearrange("(kt p) n -> p kt n", p=P)
                for ch in range(N // NCH_A):
                    tx = txp.tile([P, KT, NCH_A], f32r)
                    nc.sync.dma_start(
                        tx[:], textT_ap[:, :, ch * NCH_A : (ch + 1) * NCH_A]
                    )
                    for dt in range(KT):
                        ps = psA.tile([P, NCH_A], f32)
                        for kt in range(KT):
                            nc.tensor.matmul(
                                ps[:],
                                weslab[:, kt, dt * P : (dt + 1) * P],
                                tx[:, kt],
                                start=(kt == 0),
                                stop=(kt == KT - 1),
                            )
                        nc.vector.tensor_copy(
                            tempT[:, dt, ch * NCH_A : (ch + 1) * NCH_A], ps[:]
                        )

            # ---- Phase B: scoresT per m-tile + softmax over n, spill fp8
            # hi/lo attn per m-pair ----
            with ExitStack() as b_stack:
              if "B" in phases:
                ctp = b_stack.enter_context(tc.tile_pool(name="ctp", bufs=CTP_BUFS))
                psB = b_stack.enter_context(
                    tc.tile_pool(name="psB", bufs=PSB_BUFS, space="PSUM")
                )
                smp = b_stack.enter_context(tc.tile_pool(name="smp", bufs=4))
                epl = b_stack.enter_context(tc.tile_pool(name="epl", bufs=EPL_BUFS))
                NJ = 512
                for mt in range(MT):
                    pair, half = mt // 2, mt % 2
                    ctm = ctp.tile([P, KT, P], f32r)
                    nc.sync.dma_start(ctm[:], ctxT_d[mt])
                    psw = psB.tile([P, N], f32, tag="psB", name="psB")
                    for j in range(N // NJ):
                        for kt in range(KT):
                            nc.tensor.matmul(
                                psw[:, j * NJ : (j + 1) * NJ],
                                ctm[:, kt],
                                tempT[:, kt, j * NJ : (j + 1) * NJ],
                                start=(kt == 0),
                                stop=(kt == KT - 1),
                            )
                    nmax = smp.tile([P, 1], f32)
                    nc.vector.reduce_max(
                        nmax[:], psw[:], axis=mybir.AxisListType.X, negate=True
                    )
                    esb = epl.tile([P, N], f32)
                    z = smp.tile([P, 1], f32)
                    nc.scalar.activation(
                        esb[:],
                        psw[:],
                        mybir.ActivationFunctionType.Exp,
                        bias=nmax[:],
                        accum_out=z[:],
                    )
                    zinv = smp.tile([P, 1], f32)
                    nc.vector.reciprocal(zinv[:], z[:])
                    zs = smp.tile([P, 1], f32)
                    nc.vector.tensor_scalar_mul(zs[:], zinv[:], ALPHA)
                    if half == 0:
                        ah = atp.tile([P, 2, N], f8)
                        al = atp.tile([P, 2, N], f8)
                        keep_pairs[pair] = (ah, al)
                    ah, al = keep_pairs[pair]
                    # hi = f8(esb * (ALPHA/Z)) on ACT; lo = f8(same - hi) on DVE
                    nc.scalar.activation(
                        ah[:, half],
                        esb[:],
                        mybir.ActivationFunctionType.Copy,
                        scale=zs[:],
                    )
                    nc.vector.scalar_tensor_tensor(
                        al[:, half], esb[:], zs[:], ah[:, half], mult, subtract
                    )
                    if half == 1 and pair < MT2 - KEEP_PAIRS:
                        nc.sync.dma_start(attnh_sc[pair], ah[:])
                        nc.sync.dma_start(attnl_sc[pair], al[:])

        # ---- Phase C+D: resT accum over m (fp8 DoubleRow), then
        # out = resT.T @ (32*W2T) (fp8 DoubleRow) ----
        with ExitStack() as cd_stack:
          if "C" in phases:
            DR = mybir.MatmulPerfMode.DoubleRow
            acp = cd_stack.enter_context(tc.tile_pool(name="acp", bufs=ACP_BUFS))
            rtp = cd_stack.enter_context(tc.tile_pool(name="rtp", bufs=2))
            outp = cd_stack.enter_context(tc.tile_pool(name="outp", bufs=2))
            psC = cd_stack.enter_context(
                tc.tile_pool(name="psC", bufs=PSC_BUFS, space="PSUM")
            )
            psD = cd_stack.enter_context(tc.tile_pool(name="psD", bufs=2, space="PSUM"))
            n_dram = MT2 - KEEP_PAIRS
            for ch in range(N // NCH_C):
                nsl = slice(ch * NCH_C, (ch + 1) * NCH_C)
                achh = acp.tile([P, n_dram, 2, NCH_C], f8)
                achl = acp.tile([P, n_dram, 2, NCH_C], f8)
                # per-pair loads so C's accumulation chases B's per-pair attn
                # writes instead of waiting for all of B to finish
                for mp in range(n_dram):
                    nc.sync.dma_start(achh[:, mp], attnh_sc[mp][:, :, nsl])
                    nc.sync.dma_start(achl[:, mp], attnl_sc[mp][:, :, nsl])
                resh = rtp.tile([P, DT2, 2, NCH_C], f8)
                resl = rtp.tile([P, DT2, 2, NCH_C], f8)
                for g in range(2):
                    pss = [
                        psC.tile([P, NCH_C], f32, tag="psC", name="psC")
                        for _ in range(4)
                    ]
                    for mp in range(MT2):
                        if mp < n_dram:
                            rh, rl = achh[:, mp], achl[:, mp]
                        else:
                            kh, kl = keep_pairs[mp]
                            rh, rl = kh[:, :, nsl], kl[:, :, nsl]
                        for i in range(4):
                            dtt = g * 4 + i
                            dsl = slice(dtt * P, (dtt + 1) * P)
                            lhH = ctxh_sb[:, mp, :, dsl]
                            lhL = ctxl_sb[:, mp, :, dsl]
                            nc.tensor.matmul(
                                pss[i][:], lhH, rh, start=(mp == 0), stop=False,
                                perf_mode=DR,
                            )
                            nc.tensor.matmul(
                                pss[i][:], lhH, rl, start=False, stop=False,
                                perf_mode=DR,
                            )
                            nc.tensor.matmul(
                                pss[i][:], lhL, rh, start=False,
                                stop=(mp == MT2 - 1), perf_mode=DR,
                            )
                    for i in range(4):
                        dtt = g * 4 + i
                        t4, hf = dtt // 2, dtt % 2
                        # res/16 = psC*C1; hi on ACT, lo fused on DVE
                        nc.scalar.activation(
                            resh[:, t4, hf],
                            pss[i][:],
                            mybir.ActivationFunctionType.Copy,
                            scale=C1,
                        )
                        nc.vector.scalar_tensor_tensor(
                            resl[:, t4, hf], pss[i][:], C1, resh[:, t4, hf],
                            mult, subtract,
                        )
                for nb in range(NCH_C // P if "D" in phases else 0):
                    osb = outp.tile([P, D], f32)
                    bsl = slice(nb * P, (nb + 1) * P)
                    for dc in range(2):
                        ps = psD.tile([P, 512], f32, tag="psD", name="psD")
                        csl = slice(dc * 512, (dc + 1) * 512)
                        for t4 in range(DT2):
                            lhH = resh[:, t4, :, bsl]
                            lhL = resl[:, t4, :, bsl]
                            rhH = w2h_sb[:, t4, :, csl]
                            rhL = w2l_sb[:, t4, :, csl]
                            nc.tensor.matmul(
                                ps[:], lhH, rhH, start=(t4 == 0), stop=False,
                                perf_mode=DR,
                            )
                            nc.tensor.matmul(
                                ps[:], lhH, rhL, start=False, stop=False,
                                perf_mode=DR,
                            )
                            nc.tensor.matmul(
                                ps[:], lhL, rhH, start=False,
                                stop=(t4 == DT2 - 1), perf_mode=DR,
                            )
                        # psD = 2*out -> evict with scale 0.5 on ACT
                        nc.scalar.activation(
                            osb[:, csl],
                            ps[:],
                            mybir.ActivationFunctionType.Copy,
                            scale=DOUT,
                        )
                    row0 = ch * NCH_C + nb * P
                    nc.scalar.dma_start(out_d[:][row0 : row0 + P, :], osb[:])


def _prep_inputs(text, context, We, W2):
    """Per-core host-side shard + transpose + fp32r/fp8 pre-round."""
    we_r = r11(We)
    w2s = r11(W2.T).astype(np.float32) * np.float32(W2S)
    w2h, w2l = _split8(w2s)
    w2h_t = _pair_layout(w2h.astype(np.float32), DT2).astype(ml_dtypes.float8_e4m3)
    w2l_t = _pair_layout(w2l.astype(np.float32), DT2).astype(ml_dtypes.float8_e4m3)
    maps = []
    for b in range(B):
        cb = np.ascontiguousarray(context[b], dtype=np.float32)
        ch, cl = _split8(cb)
        maps.append(
            {
                "textT": r11(text[b].T),
                "ctxT": np.ascontiguousarray(
                    r11(cb.T).reshape(KT, P, MT, P).transpose(2, 1, 0, 3)
                ),
                "ctxh": _pair_layout(ch.astype(np.float32), MT2).astype(
                    ml_dtypes.float8_e4m3
                ),
                "ctxl": _pair_layout(cl.astype(np.float32), MT2).astype(
                    ml_dtypes.float8_e4m3
                ),
                "we": we_r,
                "w2h": w2h_t,
                "w2l": w2l_t,
            }
        )
    return maps


def kernel(text, context, We, W2, _trace=False):
    from concourse.bass_utils import run_bass_kernel_spmd

    if "nc" not in _cache:
        _cache["nc"] = _build()
    nc = _cache["nc"]
    in_maps = _prep_inputs(
        np.asarray(text), np.asarray(context), np.asarray(We), np.asarray(W2)
    )
    res = run_bass_kernel_spmd(nc, in_maps, list(range(B)), trace=_trace)
    out = np.stack([res.results[c]["out"] for c in range(B)])
    if _trace:
        return out, res
    return out
